# revision 1
# baseline (speedup 1.0000x reference)
"""Nose-Hoover checkpointed integrator on 8 Trainium2 cores.

Data-parallel: 4096 systems sharded as 512 systems/core; each core integrates
its shard for n_steps, storing (x, v) snapshots every store_every steps.

Per-core layout: [128 partitions = systems (s mod 128), free = G groups of
64 dof], group g = s // 128 (G = 4), split into 2 phase-shifted streams of
2 groups each. Per-system scalars live as [128, 2] tiles per stream.

Math (per step, force = -x):
  beta := -(DT/2)*alpha (thermostat factor f = exp(beta)); s := -DT^2/(8Q);
  each alpha update adds s*v2 to beta plus a compile-time constant q folded
  into per-position Exp biases (renormalized once per loop iteration).

  With RT = s*v2^a and B = beta~ after update 1:
    F = exp(B + u1*q), F2 = exp(2B + 2*u1*q)
    update 2:  B2 = B + RT*F2
    TTR:       B3[:,g] = B2[:,g] + sum(s*V_g^2)  (square+reduce+update3 fused)
    F2' = exp(2*B3 + 2*u3*q), FP = exp(B3 + u3*q)
    RTc = B3 - B2;  RT' = RTc*F2'
    updates 4 and 1' fuse:  B' = B3 + 2*RT'
  V is kept without its trailing thermostat factor; GF = FP_prev * F applies
  both pending factors in one multiply before the kick-drift-kick.

Engine split per step per stream: ACT: 4 Exps. Pool: GF/RT/B updates and the
V-scale. DVE: kick-drift-kick STTs + 2 TTRs. The two streams are emitted
phase-shifted so one stream's thermostat (ACT/Pool) overlaps the other's
DVE segment.
"""

import numpy as np

DT = 0.001
N_CORES = 8
P = 128

_BUILD_CACHE = {}


def _elide_redundant_waits(nc, mybir):
    """Drop sem-waits already implied by an earlier wait on the same engine
    within the same basic block (sem values are monotone within a block;
    resets happen in the loop's reset block)."""
    for f in nc.m.functions:
        # Semaphores that are ever decremented are not monotone; skip them.
        nonmono = set()
        for bb in f.blocks:
            for inst in bb.instructions:
                si = inst.sync_info
                if si is None:
                    continue
                for u in si.on_update:
                    if u.update_mode not in ("sem-inc", "sem-add-imm") or (
                        u.update_value is not None and u.update_value < 0
                    ):
                        nonmono.add((u.sync_type, u.id))
        for bb in f.blocks:
            seen = {}  # (engine, sync_type, sem id) -> max value waited for
            for inst in bb.instructions:
                si = inst.sync_info
                if si is None or not si.on_wait:
                    continue
                kept = []
                for w in si.on_wait:
                    if (
                        w.wait_value is None
                        or w.wait_reg is not None
                        or w.wait_mode != "sem-ge-imm"
                        or (w.sync_type, w.id) in nonmono
                    ):
                        kept.append(w)
                        continue
                    key = (inst.engine, w.sync_type, w.id)
                    if seen.get(key, -1) >= w.wait_value:
                        continue
                    seen[key] = w.wait_value
                    kept.append(w)
                if len(kept) != len(si.on_wait):
                    inst.sync_info = mybir.SyncInfo(
                        on_wait=kept, on_update=list(si.on_update)
                    )


def _split_multi_waits(nc, mybir):
    """This container's walrus encodes at most one sem-wait per instruction;
    hoist extra waits onto single-wait NoOps on the same engine."""
    for f in nc.m.functions:
        for bb in f.blocks:
            out = []
            for inst in bb.instructions:
                si = inst.sync_info
                if si is not None and len(si.on_wait) > 1:
                    waits = list(si.on_wait)
                    for w in waits[:-1]:
                        out.append(
                            mybir.InstNoOp(
                                name=nc.get_next_instruction_name(),
                                sync_info=mybir.SyncInfo(on_wait=[w], on_update=[]),
                                bass_nofuse=True,
                                engine=inst.engine,
                            )
                        )
                    inst.sync_info = mybir.SyncInfo(
                        on_wait=[waits[-1]], on_update=list(si.on_update)
                    )
                out.append(inst)
            bb.instructions = out


def _build_v3(
    B_core,
    D,
    n_steps,
    store_every,
    kT,
    mass,
    Q,
    bench_iters=None,
    n_streams=2,
    chunks_per_iter=4,
):
    import concourse.bass as bass
    import concourse.mybir as mybir
    from concourse.tile import TileContext

    G_total = B_core // P
    GH = G_total // n_streams
    FDH = GH * D
    n_chunks = n_steps // store_every
    assert n_steps == n_chunks * store_every
    if bench_iters is not None:
        n_chunks = chunks_per_iter
    while n_chunks % chunks_per_iter:
        chunks_per_iter -= 1
    steps_per_iter = store_every * chunks_per_iter

    k = DT / (2.0 * mass)
    e = float(D) * kT
    s = -(DT * DT) / (8.0 * Q)
    q = -s * e
    m = -DT / 2.0

    AF = mybir.ActivationFunctionType
    OP = mybir.AluOpType
    f32 = mybir.dt.float32

    nc = bass.Bass()

    def reg_const(val):
        key = (f32, float(val))
        if key not in nc.const_aps.aps:
            t = nc.alloc_sbuf_tensor(f"constb-{len(nc.const_aps.aps)}", [128, 1], f32)
            nc.gpsimd.memset(t.ap(), float(val))
            nc.const_aps.aps[key] = t.ap()

    for p_pos in range(steps_per_iter):
        for u in (4 * p_pos + 1, 4 * p_pos + 3):
            reg_const(u * q)
            reg_const(2 * u * q)
    reg_const(0.0)
    nc.all_engine_barrier()

    x0 = nc.dram_tensor("x0", [B_core, D], f32, kind="ExternalInput")
    v0 = nc.dram_tensor("v0", [B_core, D], f32, kind="ExternalInput")
    a0 = nc.dram_tensor("alpha0", [B_core], f32, kind="ExternalInput")
    n_loop_out = n_chunks // chunks_per_iter if bench_iters is None else 1
    # The dynamic-slot DMA lowering only supports a bare loop register as the
    # index, so the output is [n_loop, chunks_per_iter, ...] with ds(ci) on
    # dim 0 and the static chunk-position j on dim 1.
    out_x = nc.dram_tensor(
        "out_x",
        [n_loop_out, chunks_per_iter, G_total, P, D],
        f32,
        kind="ExternalOutput",
    )
    out_v = nc.dram_tensor(
        "out_v",
        [n_loop_out, chunks_per_iter, G_total, P, D],
        f32,
        kind="ExternalOutput",
    )

    with TileContext(nc) as tc:
        with (
            tc.tile_pool(name="state", bufs=1) as state,
            tc.tile_pool(name="stage", bufs=3) as stage,
        ):
            lanes = []
            for li in range(n_streams):
                t = lambda shape, nm: state.tile(
                    shape, f32, tag=f"{nm}{li}", name=f"{nm}{li}"
                )
                ln = {
                    "X": t([P, FDH], "X"),
                    "V": t([P, FDH], "V"),
                    "SQ": t([P, FDH], "SQ"),
                    "B": t([P, GH], "B"),
                    "B2": t([P, GH], "B2"),
                    "B3": t([P, GH], "B3"),
                    "RT": t([P, GH], "RT"),
                    "RT2": t([P, GH], "RT2"),
                    "RTC": t([P, GH], "RTC"),
                    "F": t([P, GH], "F"),
                    "F2": t([P, GH], "F2"),
                    "FP": t([P, GH], "FP"),
                    "F2P": t([P, GH], "F2P"),
                    "GF": t([P, GH], "GF"),
                    "T1": t([P, GH], "T1"),
                    "R": t([P, GH], "R"),
                    "g0": li * GH,
                    "sq_act": li == 1,
                }
                lanes.append(ln)

            def gsl(g):
                return slice(g * D, (g + 1) * D)

            for ln in lanes:
                g0 = ln["g0"]
                nc.sync.dma_start(
                    out=ln["X"][:].rearrange("p (g d) -> p g d", g=GH),
                    in_=x0[:].rearrange("(g p) d -> p g d", p=P)[:, g0 : g0 + GH, :],
                )
                nc.sync.dma_start(
                    out=ln["V"][:].rearrange("p (g d) -> p g d", g=GH),
                    in_=v0[:].rearrange("(g p) d -> p g d", p=P)[:, g0 : g0 + GH, :],
                )
                nc.sync.dma_start(
                    out=ln["T1"][:],
                    in_=a0[:].rearrange("(g p) -> p g", p=P)[:, g0 : g0 + GH],
                )
            for ln in lanes:
                nc.vector.tensor_scalar(ln["B2"][:], ln["T1"][:], m, None, OP.mult)
                nc.vector.memset(ln["FP"][:], 1.0)
                nc.vector.tensor_tensor(ln["SQ"][:], ln["V"][:], ln["V"][:], OP.mult)
                nc.vector.tensor_reduce(
                    out=ln["R"][:],
                    in_=ln["SQ"][:].rearrange("p (g d) -> p g d", g=GH),
                    axis=mybir.AxisListType.X,
                    op=OP.add,
                )
                nc.vector.tensor_scalar(ln["RT"][:], ln["R"][:], s, None, OP.mult)
                nc.vector.tensor_tensor(ln["B"][:], ln["RT"][:], ln["B2"][:], OP.add)

            def S1e(ln, p):
                u1 = 4 * p + 1
                nc.scalar.activation(
                    out=ln["F"][:], in_=ln["B"][:], func=AF.Exp, bias=u1 * q
                )
                nc.scalar.activation(
                    out=ln["F2"][:],
                    in_=ln["B"][:],
                    func=AF.Exp,
                    scale=2.0,
                    bias=2 * u1 * q,
                )

            def S1d(ln, p):
                nc.vector.tensor_tensor(ln["GF"][:], ln["FP"][:], ln["F"][:], OP.mult)
                nc.vector.tensor_tensor(ln["RT2"][:], ln["RT"][:], ln["F2"][:], OP.mult)
                nc.vector.tensor_tensor(ln["B2"][:], ln["B"][:], ln["RT2"][:], OP.add)

            def S2d(ln, p):
                V, X = ln["V"], ln["X"]
                for g in range(GH):
                    nc.vector.tensor_scalar(
                        V[:, gsl(g)], V[:, gsl(g)], ln["GF"][:, g : g + 1], None, OP.mult
                    )
                nc.vector.scalar_tensor_tensor(V[:], X[:], -k, V[:], OP.mult, OP.add)
                nc.vector.scalar_tensor_tensor(X[:], V[:], DT, X[:], OP.mult, OP.add)
                nc.vector.scalar_tensor_tensor(V[:], X[:], -k, V[:], OP.mult, OP.add)

            def S2a(ln, p):
                if ln["sq_act"]:
                    # ACT squares with accumulate (frees DVE on this lane)
                    for g in range(GH):
                        nc.scalar.activation(
                            out=ln["SQ"][:, gsl(g)],
                            in_=ln["V"][:, gsl(g)],
                            func=AF.Square,
                            accum_out=ln["R"][:, g : g + 1],
                        )
                else:
                    # squares + per-group reduce on DVE: kick2 -> squares ->
                    # update3 runs without a cross-engine hop
                    nc.vector.tensor_tensor(
                        ln["SQ"][:], ln["V"][:], ln["V"][:], OP.mult
                    )
                    nc.vector.tensor_reduce(
                        out=ln["R"][:],
                        in_=ln["SQ"][:].rearrange("p (g d) -> p g d", g=GH),
                        axis=mybir.AxisListType.X,
                        op=OP.add,
                    )

            def S3d1(ln, p):
                # update 3: B3 = B2 + s*R
                nc.vector.tensor_scalar(ln["RTC"][:], ln["R"][:], s, None, OP.mult)
                nc.vector.tensor_tensor(ln["B3"][:], ln["RTC"][:], ln["B2"][:], OP.add)

            def S3e(ln, p):
                u3 = 4 * p + 3
                nc.scalar.activation(
                    out=ln["F2P"][:],
                    in_=ln["B3"][:],
                    func=AF.Exp,
                    scale=2.0,
                    bias=2 * u3 * q,
                )
                nc.scalar.activation(
                    out=ln["FP"][:], in_=ln["B3"][:], func=AF.Exp, bias=u3 * q
                )

            def S3d2(ln, p):
                nc.vector.tensor_tensor(ln["RT"][:], ln["RTC"][:], ln["F2P"][:], OP.mult)
                nc.vector.scalar_tensor_tensor(
                    ln["B"][:], ln["RT"][:], 2.0, ln["B3"][:], OP.mult, OP.add
                )

            def snap(ln, snap_ci):
                if snap_ci is not None:
                    cj, slot = snap_ci
                    g0 = ln["g0"]
                    XS = stage.tile([P, FDH], f32, tag=f"XS{g0}", name=f"XS{g0}")
                    VS = stage.tile([P, FDH], f32, tag=f"VS{g0}", name=f"VS{g0}")
                    nc.gpsimd.tensor_copy(XS[:], ln["X"][:])
                    for g in range(GH):
                        nc.scalar.activation(
                            out=VS[:, gsl(g)],
                            in_=ln["V"][:, gsl(g)],
                            func=AF.Copy,
                            scale=ln["FP"][:, g : g + 1],
                        )
                    nc.sync.dma_start(
                        out=out_x[slot, cj : cj + 1, g0 : g0 + GH, :, :].rearrange(
                            "o c g p d -> (o c p) g d"
                        ),
                        in_=XS[:].rearrange("p (g d) -> p g d", g=GH),
                    )
                    nc.sync.dma_start(
                        out=out_v[slot, cj : cj + 1, g0 : g0 + GH, :, :].rearrange(
                            "o c g p d -> (o c p) g d"
                        ),
                        in_=VS[:].rearrange("p (g d) -> p g d", g=GH),
                    )

            def renorm(ln):
                nc.vector.tensor_scalar(
                    ln["B"][:], ln["B"][:], 4.0 * steps_per_iter * q, None, OP.add
                )

            n_loop = (
                n_chunks // chunks_per_iter if bench_iters is None else bench_iters
            )
            SPI = steps_per_iter

            def snap_slot(ci, p):
                if (p + 1) % store_every:
                    return None
                cj = p // store_every
                if bench_iters is None:
                    return cj, bass.ds(ci, 1)
                return cj, slice(0, 1)

            def H1(ln, p):
                # DVE-heavy front half: GF/RT2/B2 smalls + V-scale + kicks
                S1d(ln, p)
                S2d(ln, p)

            def H2(ln, p, ci):
                # ACT-heavy tail half: squares + exps (+ snapshot) + smalls
                S2a(ln, p)
                S3d1(ln, p)
                S3e(ln, p)
                snap(ln, snap_slot(ci, p))
                S3d2(ln, p)
                if p + 1 < SPI:
                    S1e(ln, p + 1)
                else:
                    renorm(ln)

            if n_loop > 0:
                A, Bl = lanes[0], lanes[1]
                with tc.For_i(
                    0, n_loop, hint_engines=(mybir.EngineType.DVE,)
                ) as ci:
                    # Half-step phase shift: lane B's ACT-heavy tail executes
                    # while lane A's DVE-heavy front occupies DVE, and vice
                    # versa.
                    S1e(A, 0)
                    S1e(Bl, 0)
                    for p in range(SPI):
                        H1(A, p)
                        if p > 0:
                            H2(Bl, p - 1, ci)
                        H2(A, p, ci)
                        H1(Bl, p)
                    H2(Bl, SPI - 1, ci)

    _elide_redundant_waits(nc, mybir)
    _split_multi_waits(nc, mybir)
    return nc


def kernel(x0, v0, alpha0, kT, mass, Q, n_steps, store_every):
    from concourse.bass_utils import run_bass_kernel_spmd

    x0 = np.asarray(x0, dtype=np.float32)
    v0 = np.asarray(v0, dtype=np.float32)
    alpha0 = np.asarray(alpha0, dtype=np.float32)
    kT_f = float(np.asarray(kT))
    mass_f = float(np.asarray(mass))
    Q_f = float(np.asarray(Q))
    n_steps = int(np.asarray(n_steps))
    store_every = int(np.asarray(store_every))

    B, D = x0.shape
    B_core = B // N_CORES
    n_chunks = n_steps // store_every

    key = (B_core, D, n_steps, store_every, kT_f, mass_f, Q_f)
    if key not in _BUILD_CACHE:
        _BUILD_CACHE[key] = _build_v3(
            B_core, D, n_steps, store_every, kT_f, mass_f, Q_f
        )
    nc = _BUILD_CACHE[key]

    in_maps = []
    for c in range(N_CORES):
        sl = slice(c * B_core, (c + 1) * B_core)
        in_maps.append(
            {
                "x0": np.ascontiguousarray(x0[sl]),
                "v0": np.ascontiguousarray(v0[sl]),
                "alpha0": np.ascontiguousarray(alpha0[sl]),
            }
        )

    res = run_bass_kernel_spmd(nc, in_maps, core_ids=list(range(N_CORES)))
    results = res.results

    traj_x = np.empty((n_chunks + 1, B, D), np.float32)
    traj_v = np.empty((n_chunks + 1, B, D), np.float32)
    traj_x[0] = x0
    traj_v[0] = v0
    for c in range(N_CORES):
        sl = slice(c * B_core, (c + 1) * B_core)
        traj_x[1:, sl] = results[c]["out_x"].reshape(n_chunks, B_core, D)
        traj_v[1:, sl] = results[c]["out_v"].reshape(n_chunks, B_core, D)
    return traj_x, traj_v



# revision 20
# speedup vs baseline: 1.2706x; 1.2706x over previous
"""Nose-Hoover checkpointed integrator on 8 Trainium2 cores.

Data-parallel: 4096 systems sharded as 512 systems/core; each core integrates
its shard for n_steps, storing (x, v) snapshots every store_every steps.

Per-core layout: ONE stream, [128 partitions = systems (s mod 128), free =
4 groups of 64 dof], group g = s // 128. Per-system scalars are [128, 4].

Math (per step, force = -x, dt=DT):
  beta := -(DT/2)*alpha; s := -DT^2/(8Q); q := -s*D*kT.
  With the O(beta) factors on the s*v2 recurrence terms Taylor-dropped
  (|error| ~ 3e-8/update), the 4 alpha updates collapse to ONE doubled state
  var D2 := 2*beta2:
      D2(p+1) = D2(p) + 8*s*R(p)            (R = |v|^2 after the KDK)
  and the two thermostat scales per step combine into ONE factor
      GF(p+2) = exp(D2(p-1) + 20*s*R(p-1) + bias)  [R one step stale, ~6e-8]
  computed during step p -> the reduce->exp->scale path has ~2 steps of
  slack and is OFF the critical chain. Biases (q-counts, and the a^SPI
  V-renorm fold at the iteration wrap) are compile-time per-position
  constants served from an iota-built [128, SPI] table.

  Kick-drift-kick with force -x is linear; storing X scaled by a^-p/c and V
  by a^-p (a = 1-k*DT, c = DT/a) makes the drift a plain add:
      Vs = GF*V (one broadcast TT);  V' = (b*DT/a^2)*X + Vs;  X' = Vs + X
  with an a^SPI renorm once per loop iteration (exact algebra).

Per step: 8 DVE instructions (Vs, TR, V', A, D2, Xn, SQ halves), 1 ACT exp.
Emission interleaves step p's front with step p-1's reduce/thermostat so
every same-engine RAW pair has >=1 independent instruction between producer
and consumer (HW per-instruction cost ~220-300ns dominates; measured via
microbenchmarks -- Pool and ACT run 2-10x the cost model's guess on real
silicon, so everything hot lives on DVE).
"""

import math

import numpy as np

DT = 0.001
N_CORES = 8
P = 128

_BUILD_CACHE = {}

CFG_XN = "dve"  # "dve" | "pool": engine for the X drift add


def _pick_cpi(n_chunks, store_every, cpi=20):
    # largest cpi <= 20 dividing n_chunks with steps-per-iter a multiple of 4
    while cpi > 1 and ((n_chunks % cpi) or ((store_every * cpi) % 4)):
        cpi -= 1
    return cpi


def _elide_redundant_waits(nc, mybir):
    """Drop sem-waits already implied by an earlier wait on the same engine
    within the same basic block (sem values are monotone within a block;
    resets happen in the loop's reset block)."""
    for f in nc.m.functions:
        # Semaphores that are ever decremented are not monotone; skip them.
        nonmono = set()
        for bb in f.blocks:
            for inst in bb.instructions:
                si = inst.sync_info
                if si is None:
                    continue
                for u in si.on_update:
                    if u.update_mode not in ("sem-inc", "sem-add-imm") or (
                        u.update_value is not None and u.update_value < 0
                    ):
                        nonmono.add((u.sync_type, u.id))
        for bb in f.blocks:
            seen = {}  # (engine, sync_type, sem id) -> max value waited for
            for inst in bb.instructions:
                si = inst.sync_info
                if si is None or not si.on_wait:
                    continue
                kept = []
                for w in si.on_wait:
                    if (
                        w.wait_value is None
                        or w.wait_reg is not None
                        or w.wait_mode != "sem-ge-imm"
                        or (w.sync_type, w.id) in nonmono
                    ):
                        kept.append(w)
                        continue
                    key = (inst.engine, w.sync_type, w.id)
                    if seen.get(key, -1) >= w.wait_value:
                        continue
                    seen[key] = w.wait_value
                    kept.append(w)
                if len(kept) != len(si.on_wait):
                    inst.sync_info = mybir.SyncInfo(
                        on_wait=kept, on_update=list(si.on_update)
                    )


def _split_multi_waits(nc, mybir):
    """This container's walrus encodes at most one sem-wait per instruction;
    hoist extra waits onto single-wait NoOps on the same engine."""
    for f in nc.m.functions:
        for bb in f.blocks:
            out = []
            for inst in bb.instructions:
                si = inst.sync_info
                if si is not None and len(si.on_wait) > 1:
                    waits = list(si.on_wait)
                    for w in waits[:-1]:
                        out.append(
                            mybir.InstNoOp(
                                name=nc.get_next_instruction_name(),
                                sync_info=mybir.SyncInfo(on_wait=[w], on_update=[]),
                                bass_nofuse=True,
                                engine=inst.engine,
                            )
                        )
                    inst.sync_info = mybir.SyncInfo(
                        on_wait=[waits[-1]], on_update=list(si.on_update)
                    )
                out.append(inst)
            bb.instructions = out


def _build_v3(
    B_core,
    D,
    n_steps,
    store_every,
    kT,
    mass,
    Q,
    bench_iters=None,
    chunks_per_iter=20,
):
    import concourse.bass as bass
    import concourse.mybir as mybir
    from concourse.tile import TileContext

    G = B_core // P  # 4 groups of 128 systems
    FD = G * D  # 256
    H = FD // 2
    n_chunks = n_steps // store_every
    assert n_steps == n_chunks * store_every
    if bench_iters is not None:
        n_chunks = chunks_per_iter
    chunks_per_iter = _pick_cpi(n_chunks, store_every, chunks_per_iter)
    SPI = store_every * chunks_per_iter
    assert SPI % 4 == 0
    SB = chunks_per_iter // 2 if chunks_per_iter % 2 == 0 else 1
    NB = chunks_per_iter // SB

    k = DT / (2.0 * mass)
    a = 1.0 - k * DT
    ba = -k * (1.0 + a) / a
    c = DT / a
    bac = ba * c  # X stored as X/c: V' = bac*Xt + Vs; Xt' = Vs + Xt
    e = float(D) * kT
    s = -(DT * DT) / (8.0 * Q)
    q = -s * e
    m = -DT / 2.0  # beta = m*alpha
    lna = math.log(a)
    wrap_bias = 8.0 * SPI * q + SPI * lna  # GF(SPI) carries the V renorm

    AF = mybir.ActivationFunctionType
    OP = mybir.AluOpType
    f32 = mybir.dt.float32
    i32 = mybir.dt.int32

    nc = bass.Bass()

    def reg_const(val):
        key = (f32, float(val))
        if key not in nc.const_aps.aps:
            t = nc.alloc_sbuf_tensor(f"constb-{len(nc.const_aps.aps)}", [128, 1], f32)
            nc.gpsimd.memset(t.ap(), float(val))
            nc.const_aps.aps[key] = t.ap()

    for v in (0.0, q, 8.0 * q, 16.0 * q, wrap_bias):
        reg_const(v)
    nc.all_engine_barrier()

    x0 = nc.dram_tensor("x0", [B_core, D], f32, kind="ExternalInput")
    v0 = nc.dram_tensor("v0", [B_core, D], f32, kind="ExternalInput")
    a0 = nc.dram_tensor("alpha0", [B_core], f32, kind="ExternalInput")
    n_loop_out = n_chunks // chunks_per_iter if bench_iters is None else 1
    # ds(ci) dynamic slot on dim 0; [.., P, SB, G, D] keeps each partition's
    # whole snapshot batch contiguous (128-descriptor DMAs, and only 2*NB
    # dynamic DMAs per body -- more exhausts SP bounds-check registers)
    out_x = nc.dram_tensor(
        "out_x", [n_loop_out, NB, P, SB, G, D], f32, kind="ExternalOutput"
    )
    out_v = nc.dram_tensor(
        "out_v", [n_loop_out, NB, P, SB, G, D], f32, kind="ExternalOutput"
    )

    with TileContext(nc) as tc:
        with (
            tc.tile_pool(name="state", bufs=1) as state,
            tc.tile_pool(name="stage", bufs=3) as stage,
        ):
            t = lambda shape, nm, dt=f32: state.tile(shape, dt, tag=nm, name=nm)
            X = [t([P, FD], "X0"), t([P, FD], "X1")]
            V = [t([P, FD], f"V{i}") for i in range(4)]
            Vs = [t([P, FD], "Vs0"), t([P, FD], "Vs1")]
            SQ = t([P, FD], "SQ")
            RB = [t([P, G], "R0"), t([P, G], "R1")]
            D2 = t([P, G], "D2")
            Tb = t([P, G], "Tb")
            A = [t([P, G], "A0"), t([P, G], "A1")]
            GF = [t([P, G], f"GF{i}") for i in range(4)]
            W2 = t([P, G], "W2")
            FP = t([P, G], "FP")
            IOT = t([P, SPI], "IOT", i32)
            IOTF = t([P, SPI], "IOTF")
            BIAS = t([P, SPI], "BIAS")
            FPB = t([P, SPI], "FPB")

            def bcast(tile4):
                return (
                    tile4[:]
                    .rearrange("p (g u) -> p g u", u=1)
                    .broadcast_to([P, G, D])
                )

            def grp(tile):
                return tile[:].rearrange("p (g d) -> p g d", g=G)

            nc.sync.dma_start(
                out=grp(X[0]), in_=x0[:].rearrange("(g p) d -> p g d", p=P)
            )
            nc.sync.dma_start(
                out=grp(V[0]), in_=v0[:].rearrange("(g p) d -> p g d", p=P)
            )
            nc.sync.dma_start(out=Tb[:], in_=a0[:].rearrange("(g p) -> p g", p=P))

            # --- bias tables: col j -> GF bias 8(j+3)q; FP bias
            # (4j+3)q + (j+1)ln(a) -----------------------------------------
            nc.gpsimd.iota(IOT[:], pattern=[[1, SPI]], base=0, channel_multiplier=0)
            nc.vector.tensor_copy(IOTF[:], IOT[:])
            nc.vector.tensor_scalar(
                BIAS[:], IOTF[:], 8.0 * q, 24.0 * q, OP.mult, OP.add
            )
            nc.vector.tensor_scalar(
                FPB[:], IOTF[:], 4.0 * q + lna, 3.0 * q + lna, OP.mult, OP.add
            )

            # --- prologue: r0, D2(0), GF(0..2), X rescale ------------------
            nc.vector.tensor_scalar(X[0][:], X[0][:], 1.0 / c, None, OP.mult)
            R = RB[1]
            nc.vector.tensor_tensor(SQ[:], V[0][:], V[0][:], OP.mult)
            nc.vector.tensor_reduce(
                out=R[:], in_=grp(SQ), axis=mybir.AxisListType.X, op=OP.add
            )
            nc.vector.tensor_scalar(Tb[:], Tb[:], m, None, OP.mult)  # beta0
            nc.vector.scalar_tensor_tensor(A[0][:], R[:], s, Tb[:], OP.mult, OP.add)
            nc.scalar.activation(out=GF[0][:], in_=A[0][:], func=AF.Exp, bias=q)
            nc.vector.tensor_scalar(Tb[:], Tb[:], 2.0, None, OP.mult)  # 2*beta0
            nc.vector.scalar_tensor_tensor(
                D2[:], R[:], 4.0 * s, Tb[:], OP.mult, OP.add
            )  # D2(0)
            nc.vector.scalar_tensor_tensor(
                A[1][:], R[:], 4.0 * s, D2[:], OP.mult, OP.add
            )
            nc.scalar.activation(out=GF[1][:], in_=A[1][:], func=AF.Exp, bias=8.0 * q)
            nc.vector.scalar_tensor_tensor(
                W2[:], R[:], 12.0 * s, D2[:], OP.mult, OP.add
            )
            nc.scalar.activation(out=GF[2][:], in_=W2[:], func=AF.Exp, bias=16.0 * q)

            n_loop = n_chunks // chunks_per_iter if bench_iters is None else bench_iters

            def snap_slot(ci, j):
                if (j + 1) % store_every:
                    return None
                cj = j // store_every
                if bench_iters is None:
                    return cj, bass.ds(ci, 1)
                return cj, slice(0, 1)

            cur_stage = {}
            xn_eng = nc.gpsimd if CFG_XN == "pool" else nc.vector

            def tail_block(ci, p):
                """Thermostat + snapshot for step j = p-1, interleaved into
                cycle p (one-step R lag keeps GF(j+3) two steps ahead of its
                consumer)."""
                j = p - 1
                Rj = RB[j % 2]
                sc = snap_slot(ci, j)
                # A(j) = D2(j) + 20*s*R(j) -> GF(j+3); reads D2 pre-update
                nc.vector.scalar_tensor_tensor(
                    A[j % 2][:], Rj[:], 20.0 * s, D2[:], OP.mult, OP.add
                )
                if sc is not None:
                    # beta3(j) = 0.5*(D2(j) + 2*s*R(j)); exp applies scale=0.5
                    nc.vector.scalar_tensor_tensor(
                        W2[:], Rj[:], 2.0 * s, D2[:], OP.mult, OP.add
                    )
                nc.vector.scalar_tensor_tensor(
                    D2[:], Rj[:], 8.0 * s, D2[:], OP.mult, OP.add
                )
                gf_bias = wrap_bias if j + 3 == SPI else BIAS[:, j : j + 1]
                nc.scalar.activation(
                    out=GF[(j + 3) % 4][:], in_=A[j % 2][:], func=AF.Exp, bias=gf_bias
                )
                if sc is not None:
                    nc.scalar.activation(
                        out=FP[:],
                        in_=W2[:],
                        func=AF.Exp,
                        scale=0.5,
                        bias=FPB[:, j : j + 1],
                    )

            def stage_block(ci, p):
                # XS/VS staging + DMA for step j = p-1, emitted at the END of
                # cycle p so the DVE VS-multiply doesn't sit waiting on ACT's
                # FP exp (ACT runs GF then FP during the cycle's middle)
                j = p - 1
                sc = snap_slot(ci, j)
                if sc is None:
                    return
                Vnj = V[(j + 1) % 4]
                Xnj = X[1 - (j % 2)]
                cj, slot = sc
                bj, si = divmod(cj, SB)
                if si == 0:
                    cur_stage["x"] = stage.tile([P, SB * FD], f32, tag="XS", name="XS")
                    cur_stage["v"] = stage.tile([P, SB * FD], f32, tag="VS", name="VS")
                XS, VS = cur_stage["x"], cur_stage["v"]
                o = si * FD
                nc.vector.tensor_scalar(
                    XS[:, o : o + FD], Xnj[:], c * a ** (j + 1), None, OP.mult
                )
                nc.vector.tensor_tensor(
                    VS[:, o : o + FD].rearrange("p (g d) -> p g d", g=G),
                    grp(Vnj),
                    bcast(FP),
                    OP.mult,
                )
                if si == SB - 1:
                    nc.sync.dma_start(
                        out=out_x[slot, bj : bj + 1, :, :, :, :].rearrange(
                            "o b p s g d -> (o b p) (s g d)"
                        ),
                        in_=XS[:],
                    )
                    nc.sync.dma_start(
                        out=out_v[slot, bj : bj + 1, :, :, :, :].rearrange(
                            "o b p s g d -> (o b p) (s g d)"
                        ),
                        in_=VS[:],
                    )

            def body(ci):
                for p in range(SPI):
                    pe = p % 2
                    Xo, Xn = X[pe], X[1 - pe]
                    Vo, Vn = V[p % 4], V[(p + 1) % 4]
                    Vp = Vs[pe]
                    # thermostat scale: one broadcast multiply
                    nc.vector.tensor_tensor(
                        grp(Vp), grp(Vo), bcast(GF[p % 4]), OP.mult
                    )
                    if p > 0:
                        # reduce for step p-1 (also pads the Vs->V' RAW)
                        nc.vector.tensor_reduce(
                            out=RB[(p - 1) % 2][:],
                            in_=grp(SQ),
                            axis=mybir.AxisListType.X,
                            op=OP.add,
                        )
                    nc.vector.scalar_tensor_tensor(
                        Vn[:], Xo[:], bac, Vp[:], OP.mult, OP.add
                    )
                    if p > 0:
                        tail_block(ci, p)
                    xn_eng.tensor_tensor(Xn[:], Vp[:], Xo[:], OP.add)
                    nc.vector.tensor_tensor(
                        SQ[:, 0:H], Vn[:, 0:H], Vn[:, 0:H], OP.mult
                    )
                    nc.vector.tensor_tensor(
                        SQ[:, H:FD], Vn[:, H:FD], Vn[:, H:FD], OP.mult
                    )
                    if p > 0:
                        stage_block(ci, p)
                # close out step SPI-1 (reduce + thermostat + snapshot)
                nc.vector.tensor_reduce(
                    out=RB[(SPI - 1) % 2][:],
                    in_=grp(SQ),
                    axis=mybir.AxisListType.X,
                    op=OP.add,
                )
                tail_block(ci, SPI)
                stage_block(ci, SPI)
                # undo the per-step a^-1 store scaling on X and rebase D2
                # (V's renorm rides in GF(SPI)'s wrap bias)
                nc.vector.tensor_scalar(X[0][:], X[0][:], a**SPI, None, OP.mult)
                nc.vector.tensor_scalar(D2[:], D2[:], 8.0 * SPI * q, None, OP.add)

            if n_loop > 0:
                with tc.For_i(0, n_loop, hint_engines=(mybir.EngineType.DVE,)) as ci:
                    body(ci)

    _elide_redundant_waits(nc, mybir)
    _split_multi_waits(nc, mybir)
    return nc


def kernel(x0, v0, alpha0, kT, mass, Q, n_steps, store_every):
    from concourse.bass_utils import run_bass_kernel_spmd

    x0 = np.asarray(x0, dtype=np.float32)
    v0 = np.asarray(v0, dtype=np.float32)
    alpha0 = np.asarray(alpha0, dtype=np.float32)
    kT_f = float(np.asarray(kT))
    mass_f = float(np.asarray(mass))
    Q_f = float(np.asarray(Q))
    n_steps = int(np.asarray(n_steps))
    store_every = int(np.asarray(store_every))

    B, D = x0.shape
    B_core = B // N_CORES
    n_chunks = n_steps // store_every

    key = (B_core, D, n_steps, store_every, kT_f, mass_f, Q_f)
    if key not in _BUILD_CACHE:
        _BUILD_CACHE[key] = _build_v3(
            B_core, D, n_steps, store_every, kT_f, mass_f, Q_f
        )
    nc = _BUILD_CACHE[key]

    in_maps = []
    for cc in range(N_CORES):
        sl = slice(cc * B_core, (cc + 1) * B_core)
        in_maps.append(
            {
                "x0": np.ascontiguousarray(x0[sl]),
                "v0": np.ascontiguousarray(v0[sl]),
                "alpha0": np.ascontiguousarray(alpha0[sl]),
            }
        )

    res = run_bass_kernel_spmd(nc, in_maps, core_ids=list(range(N_CORES)))
    results = res.results

    traj_x = np.empty((n_chunks + 1, B, D), np.float32)
    traj_v = np.empty((n_chunks + 1, B, D), np.float32)
    traj_x[0] = x0
    traj_v[0] = v0
    cpi = _pick_cpi(n_chunks, store_every)
    SB = cpi // 2 if cpi % 2 == 0 else 1
    for cc in range(N_CORES):
        sl = slice(cc * B_core, (cc + 1) * B_core)
        # device layout [n_loop*NB, P, SB, G, D] -> host order (g*P + p)
        traj_x[1:, sl] = (
            results[cc]["out_x"]
            .reshape(n_chunks // SB, P, SB, B_core // P, D)
            .transpose(0, 2, 3, 1, 4)
            .reshape(n_chunks, B_core, D)
        )
        traj_v[1:, sl] = (
            results[cc]["out_v"]
            .reshape(n_chunks // SB, P, SB, B_core // P, D)
            .transpose(0, 2, 3, 1, 4)
            .reshape(n_chunks, B_core, D)
        )
    return traj_x, traj_v


# revision 21
# speedup vs baseline: 3.8682x; 3.0444x over previous
"""Nose-Hoover checkpointed integrator on 8 Trainium2 cores.

Data-parallel: 4096 systems sharded as 512 systems/core; each core integrates
its shard for n_steps, storing (x, v) snapshots every store_every steps.

Per-core layout: ONE stream, [128 partitions = systems (s mod 128), free =
4 groups of 64 dof], group g = s // 128. Per-system scalars are [128, 4].

Math (per step, force = -x, dt=DT):
  beta := -(DT/2)*alpha; s := -DT^2/(8Q); q := -s*D*kT.
  With the O(beta) factors on the s*v2 recurrence terms Taylor-dropped
  (|error| ~ 3e-8/update), the 4 alpha updates collapse to ONE doubled state
  var D2 := 2*beta2:
      D2(p+1) = D2(p) + 8*s*R(p)            (R = |v|^2 after the KDK)
  and the two thermostat scales per step combine into ONE factor
      GF(p+2) = exp(D2(p-1) + 20*s*R(p-1) + bias)  [R one step stale, ~6e-8]
  computed during step p -> the reduce->exp->scale path has ~2 steps of
  slack and is OFF the critical chain. Biases (q-counts, and the a^SPI
  V-renorm fold at the iteration wrap) are compile-time per-position
  constants served from an iota-built [128, SPI] table.

  Kick-drift-kick with force -x is linear; storing X scaled by a^-p/c and V
  by a^-p (a = 1-k*DT, c = DT/a) makes the drift a plain add:
      Vs = GF*V (one broadcast TT);  V' = (b*DT/a^2)*X + Vs;  X' = Vs + X
  with an a^SPI renorm once per loop iteration (exact algebra).

Per step: 8 DVE instructions (Vs, TR, V', A, D2, Xn, SQ halves), 1 ACT exp.
Emission interleaves step p's front with step p-1's reduce/thermostat so
every same-engine RAW pair has >=1 independent instruction between producer
and consumer (HW per-instruction cost ~220-300ns dominates; measured via
microbenchmarks -- Pool and ACT run 2-10x the cost model's guess on real
silicon, so everything hot lives on DVE).
"""

import math

import numpy as np

DT = 0.001
N_CORES = 8
P = 128

_BUILD_CACHE = {}

CFG_XN = "dve"  # "dve" | "pool": engine for the X drift add


def _pick_cpi(n_chunks, store_every, cpi=20):
    # largest cpi <= 20 dividing n_chunks with steps-per-iter a multiple of 4
    while cpi > 1 and ((n_chunks % cpi) or ((store_every * cpi) % 8)):
        cpi -= 1
    return cpi


def _elide_redundant_waits(nc, mybir):
    """Drop sem-waits already implied by an earlier wait on the same engine
    within the same basic block (sem values are monotone within a block;
    resets happen in the loop's reset block)."""
    for f in nc.m.functions:
        # Semaphores that are ever decremented are not monotone; skip them.
        nonmono = set()
        for bb in f.blocks:
            for inst in bb.instructions:
                si = inst.sync_info
                if si is None:
                    continue
                for u in si.on_update:
                    if u.update_mode not in ("sem-inc", "sem-add-imm") or (
                        u.update_value is not None and u.update_value < 0
                    ):
                        nonmono.add((u.sync_type, u.id))
        for bb in f.blocks:
            seen = {}  # (engine, sync_type, sem id) -> max value waited for
            for inst in bb.instructions:
                si = inst.sync_info
                if si is None or not si.on_wait:
                    continue
                kept = []
                for w in si.on_wait:
                    if (
                        w.wait_value is None
                        or w.wait_reg is not None
                        or w.wait_mode != "sem-ge-imm"
                        or (w.sync_type, w.id) in nonmono
                    ):
                        kept.append(w)
                        continue
                    key = (inst.engine, w.sync_type, w.id)
                    if seen.get(key, -1) >= w.wait_value:
                        continue
                    seen[key] = w.wait_value
                    kept.append(w)
                if len(kept) != len(si.on_wait):
                    inst.sync_info = mybir.SyncInfo(
                        on_wait=kept, on_update=list(si.on_update)
                    )


def _split_multi_waits(nc, mybir):
    """This container's walrus encodes at most one sem-wait per instruction;
    hoist extra waits onto single-wait NoOps on the same engine."""
    for f in nc.m.functions:
        for bb in f.blocks:
            out = []
            for inst in bb.instructions:
                si = inst.sync_info
                if si is not None and len(si.on_wait) > 1:
                    waits = list(si.on_wait)
                    for w in waits[:-1]:
                        out.append(
                            mybir.InstNoOp(
                                name=nc.get_next_instruction_name(),
                                sync_info=mybir.SyncInfo(on_wait=[w], on_update=[]),
                                bass_nofuse=True,
                                engine=inst.engine,
                            )
                        )
                    inst.sync_info = mybir.SyncInfo(
                        on_wait=[waits[-1]], on_update=list(si.on_update)
                    )
                out.append(inst)
            bb.instructions = out


def _build_v3(
    B_core,
    D,
    n_steps,
    store_every,
    kT,
    mass,
    Q,
    bench_iters=None,
    chunks_per_iter=20,
):
    import concourse.bass as bass
    import concourse.mybir as mybir
    from concourse.tile import TileContext

    G = B_core // P  # 4 groups of 128 systems
    FD = G * D  # 256
    H = FD // 2
    n_chunks = n_steps // store_every
    assert n_steps == n_chunks * store_every
    if bench_iters is not None:
        n_chunks = chunks_per_iter
    chunks_per_iter = _pick_cpi(n_chunks, store_every, chunks_per_iter)
    SPI = store_every * chunks_per_iter
    assert SPI % 8 == 0
    SB = chunks_per_iter // 2 if chunks_per_iter % 2 == 0 else 1
    NB = chunks_per_iter // SB

    k = DT / (2.0 * mass)
    a = 1.0 - k * DT
    ba = -k * (1.0 + a) / a
    c = DT / a
    bac = ba * c  # X stored as X/c: V' = bac*Xt + Vs; Xt' = Vs + Xt
    e = float(D) * kT
    s = -(DT * DT) / (8.0 * Q)
    q = -s * e
    m = -DT / 2.0  # beta = m*alpha
    lna = math.log(a)
    wrap_bias = 8.0 * SPI * q + SPI * lna  # GF(SPI) carries the V renorm

    AF = mybir.ActivationFunctionType
    OP = mybir.AluOpType
    f32 = mybir.dt.float32
    i32 = mybir.dt.int32

    nc = bass.Bass()

    def reg_const(val):
        key = (f32, float(val))
        if key not in nc.const_aps.aps:
            t = nc.alloc_sbuf_tensor(f"constb-{len(nc.const_aps.aps)}", [128, 1], f32)
            nc.gpsimd.memset(t.ap(), float(val))
            nc.const_aps.aps[key] = t.ap()

    for v in (0.0, q, 8.0 * q, 16.0 * q, 24.0 * q, wrap_bias):
        reg_const(v)
    nc.all_engine_barrier()

    x0 = nc.dram_tensor("x0", [B_core, D], f32, kind="ExternalInput")
    v0 = nc.dram_tensor("v0", [B_core, D], f32, kind="ExternalInput")
    a0 = nc.dram_tensor("alpha0", [B_core], f32, kind="ExternalInput")
    n_loop_out = n_chunks // chunks_per_iter if bench_iters is None else 1
    # ds(ci) dynamic slot on dim 0; [.., P, SB, G, D] keeps each partition's
    # whole snapshot batch contiguous (128-descriptor DMAs, and only 2*NB
    # dynamic DMAs per body -- more exhausts SP bounds-check registers)
    out_x = nc.dram_tensor(
        "out_x", [n_loop_out, NB, P, SB, G, D], f32, kind="ExternalOutput"
    )
    out_v = nc.dram_tensor(
        "out_v", [n_loop_out, NB, P, SB, G, D], f32, kind="ExternalOutput"
    )

    with TileContext(nc) as tc:
        with (
            tc.tile_pool(name="state", bufs=1) as state,
            tc.tile_pool(name="stage", bufs=3) as stage,
        ):
            t = lambda shape, nm, dt=f32: state.tile(shape, dt, tag=nm, name=nm)
            X = [t([P, FD], "X0"), t([P, FD], "X1")]
            V = [t([P, FD], f"V{i}") for i in range(4)]
            Vs = [t([P, FD], "Vs0"), t([P, FD], "Vs1")]
            SQ = t([P, FD], "SQ")
            RB = [t([P, G], "R0"), t([P, G], "R1")]
            D2 = t([P, G], "D2")
            Tb = t([P, G], "Tb")
            A4 = [t([P, 4 * G], "A40"), t([P, 4 * G], "A41")]
            GF4 = [t([P, 4 * G], "GF40"), t([P, 4 * G], "GF41")]
            W2 = t([P, G], "W2")
            FP = t([P, G], "FP")
            IOT = t([P, SPI], "IOT", i32)
            IOTF = t([P, SPI], "IOTF")
            FPB = t([P, SPI], "FPB")
            NBIA = 4 * (SPI + 4)
            IOTB = t([P, NBIA], "IOTB", i32)
            IOTBF = t([P, NBIA], "IOTBF")
            BIASB = t([P, NBIA], "BIASB")

            def bcast(tile4):
                return (
                    tile4[:]
                    .rearrange("p (g u) -> p g u", u=1)
                    .broadcast_to([P, G, D])
                )

            def grp(tile):
                return tile[:].rearrange("p (g d) -> p g d", g=G)

            nc.sync.dma_start(
                out=grp(X[0]), in_=x0[:].rearrange("(g p) d -> p g d", p=P)
            )
            nc.sync.dma_start(
                out=grp(V[0]), in_=v0[:].rearrange("(g p) d -> p g d", p=P)
            )
            nc.sync.dma_start(out=Tb[:], in_=a0[:].rearrange("(g p) -> p g", p=P))

            # --- bias tables: col j -> GF bias 8(j+3)q; FP bias
            # (4j+3)q + (j+1)ln(a) -----------------------------------------
            nc.gpsimd.iota(IOT[:], pattern=[[1, SPI]], base=0, channel_multiplier=0)
            nc.vector.tensor_copy(IOTF[:], IOT[:])
            nc.vector.tensor_scalar(
                FPB[:], IOTF[:], 4.0 * q + lna, 3.0 * q + lna, OP.mult, OP.add
            )
            # BIASB col (p', g) -> 8*p'*q (GF exp bias per step, g-repeated)
            nc.gpsimd.iota(
                IOTB[:].rearrange("p (a b) -> p a b", b=G),
                pattern=[[1, SPI + 4], [0, G]],
                base=0,
                channel_multiplier=0,
            )
            nc.vector.tensor_copy(IOTBF[:], IOTB[:])
            nc.vector.tensor_scalar(BIASB[:], IOTBF[:], 8.0 * q, None, OP.mult)

            # --- prologue: r0, D2(0), GF(0..2), X rescale ------------------
            nc.vector.tensor_scalar(X[0][:], X[0][:], 1.0 / c, None, OP.mult)
            R = RB[1]
            nc.vector.tensor_tensor(SQ[:], V[0][:], V[0][:], OP.mult)
            nc.vector.tensor_reduce(
                out=R[:], in_=grp(SQ), axis=mybir.AxisListType.X, op=OP.add
            )
            nc.vector.tensor_scalar(Tb[:], Tb[:], m, None, OP.mult)  # beta0
            nc.vector.scalar_tensor_tensor(
                A4[0][:, 0:G], R[:], s, Tb[:], OP.mult, OP.add
            )
            nc.scalar.activation(
                out=GF4[0][:, 0:G], in_=A4[0][:, 0:G], func=AF.Exp, bias=q
            )
            nc.vector.tensor_scalar(Tb[:], Tb[:], 2.0, None, OP.mult)  # 2*beta0
            nc.vector.scalar_tensor_tensor(
                D2[:], R[:], 4.0 * s, Tb[:], OP.mult, OP.add
            )  # D2(0)
            for d, coef, bv in ((1, 4.0, 8.0), (2, 12.0, 16.0), (3, 20.0, 24.0)):
                nc.vector.scalar_tensor_tensor(
                    A4[0][:, d * G : (d + 1) * G], R[:], coef * s, D2[:],
                    OP.mult, OP.add,
                )
                nc.scalar.activation(
                    out=GF4[0][:, d * G : (d + 1) * G],
                    in_=A4[0][:, d * G : (d + 1) * G],
                    func=AF.Exp,
                    bias=bv * q,
                )

            n_loop = n_chunks // chunks_per_iter if bench_iters is None else bench_iters

            def snap_slot(ci, j):
                if (j + 1) % store_every:
                    return None
                cj = j // store_every
                if bench_iters is None:
                    return cj, bass.ds(ci, 1)
                return cj, slice(0, 1)

            cur_stage = {}
            xn_eng = nc.gpsimd if CFG_XN == "pool" else nc.vector

            def tail_block(ci, p):
                """Thermostat + snapshot for step j = p-1, interleaved into
                cycle p (one-step R lag keeps GF(j+3) two steps ahead of its
                consumer)."""
                j = p - 1
                Rj = RB[j % 2]
                sc = snap_slot(ci, j)
                if p % 4 == 2:
                    # GF batch for steps p+2..p+5 from R(j), D2(j): arg(p') =
                    # D2(j) + 4*(2*(p'-j)-1)*s*R(j) + 8*p'*q. Emitted every
                    # 4th cycle -> one ACT exp and ~1 DVE wait per 4 steps.
                    mm = (p + 2) // 4
                    A4t, GF4t = A4[mm % 2], GF4[mm % 2]
                    for d in range(4):
                        nc.vector.scalar_tensor_tensor(
                            A4t[:, d * G : (d + 1) * G], Rj[:],
                            (20.0 + 8.0 * d) * s, D2[:], OP.mult, OP.add,
                        )
                if sc is not None:
                    # beta3(j) = 0.5*(D2(j) + 2*s*R(j)); exp applies scale=0.5
                    nc.vector.scalar_tensor_tensor(
                        W2[:], Rj[:], 2.0 * s, D2[:], OP.mult, OP.add
                    )
                nc.vector.scalar_tensor_tensor(
                    D2[:], Rj[:], 8.0 * s, D2[:], OP.mult, OP.add
                )
                if p % 4 == 2:
                    nc.vector.tensor_tensor(
                        A4t[:], A4t[:],
                        BIASB[:, G * (p + 2) : G * (p + 2) + 4 * G], OP.add,
                    )
                    if p + 2 == SPI:
                        # next iter's GF(0) carries the V a^SPI renorm
                        nc.vector.tensor_scalar(
                            A4t[:, 0:G], A4t[:, 0:G], SPI * lna, None, OP.add
                        )
                    nc.scalar.activation(
                        out=GF4t[:], in_=A4t[:], func=AF.Exp, bias=0.0
                    )
                if sc is not None:
                    nc.scalar.activation(
                        out=FP[:],
                        in_=W2[:],
                        func=AF.Exp,
                        scale=0.5,
                        bias=FPB[:, j : j + 1],
                    )

            def stage_block(ci, p):
                # XS/VS staging + DMA for step j = p-1, emitted at the END of
                # cycle p so the DVE VS-multiply doesn't sit waiting on ACT's
                # FP exp (ACT runs GF then FP during the cycle's middle)
                j = p - 1
                sc = snap_slot(ci, j)
                if sc is None:
                    return
                Vnj = V[(j + 1) % 4]
                Xnj = X[1 - (j % 2)]
                cj, slot = sc
                bj, si = divmod(cj, SB)
                if si == 0:
                    cur_stage["x"] = stage.tile([P, SB * FD], f32, tag="XS", name="XS")
                    cur_stage["v"] = stage.tile([P, SB * FD], f32, tag="VS", name="VS")
                XS, VS = cur_stage["x"], cur_stage["v"]
                o = si * FD
                nc.vector.tensor_scalar(
                    XS[:, o : o + FD], Xnj[:], c * a ** (j + 1), None, OP.mult
                )
                nc.vector.tensor_tensor(
                    VS[:, o : o + FD].rearrange("p (g d) -> p g d", g=G),
                    grp(Vnj),
                    bcast(FP),
                    OP.mult,
                )
                if si == SB - 1:
                    nc.sync.dma_start(
                        out=out_x[slot, bj : bj + 1, :, :, :, :].rearrange(
                            "o b p s g d -> (o b p) (s g d)"
                        ),
                        in_=XS[:],
                    )
                    nc.sync.dma_start(
                        out=out_v[slot, bj : bj + 1, :, :, :, :].rearrange(
                            "o b p s g d -> (o b p) (s g d)"
                        ),
                        in_=VS[:],
                    )

            def body(ci):
                for p in range(SPI):
                    pe = p % 2
                    Xo, Xn = X[pe], X[1 - pe]
                    Vo, Vn = V[p % 4], V[(p + 1) % 4]
                    Vp = Vs[pe]
                    # thermostat scale: one broadcast multiply
                    gfap = GF4[(p // 4) % 2][:, (p % 4) * G : (p % 4 + 1) * G]
                    nc.vector.tensor_tensor(
                        grp(Vp),
                        grp(Vo),
                        gfap.rearrange("p (g u) -> p g u", u=1).broadcast_to(
                            [P, G, D]
                        ),
                        OP.mult,
                    )
                    if p > 0:
                        # reduce for step p-1 (also pads the Vs->V' RAW)
                        nc.vector.tensor_reduce(
                            out=RB[(p - 1) % 2][:],
                            in_=grp(SQ),
                            axis=mybir.AxisListType.X,
                            op=OP.add,
                        )
                    nc.vector.scalar_tensor_tensor(
                        Vn[:], Xo[:], bac, Vp[:], OP.mult, OP.add
                    )
                    if p > 0:
                        tail_block(ci, p)
                    xn_eng.tensor_tensor(Xn[:], Vp[:], Xo[:], OP.add)
                    nc.vector.tensor_tensor(SQ[:], Vn[:], Vn[:], OP.mult)
                    if p > 0:
                        stage_block(ci, p)
                # close out step SPI-1 (reduce + thermostat + snapshot)
                nc.vector.tensor_reduce(
                    out=RB[(SPI - 1) % 2][:],
                    in_=grp(SQ),
                    axis=mybir.AxisListType.X,
                    op=OP.add,
                )
                tail_block(ci, SPI)
                stage_block(ci, SPI)
                # undo the per-step a^-1 store scaling on X and rebase D2
                # (V's renorm rides in GF(SPI)'s wrap bias)
                nc.vector.tensor_scalar(X[0][:], X[0][:], a**SPI, None, OP.mult)
                nc.vector.tensor_scalar(D2[:], D2[:], 8.0 * SPI * q, None, OP.add)

            if n_loop > 0:
                with tc.For_i(0, n_loop, hint_engines=(mybir.EngineType.DVE,)) as ci:
                    body(ci)

    _elide_redundant_waits(nc, mybir)
    _split_multi_waits(nc, mybir)
    return nc


def kernel(x0, v0, alpha0, kT, mass, Q, n_steps, store_every):
    from concourse.bass_utils import run_bass_kernel_spmd

    x0 = np.asarray(x0, dtype=np.float32)
    v0 = np.asarray(v0, dtype=np.float32)
    alpha0 = np.asarray(alpha0, dtype=np.float32)
    kT_f = float(np.asarray(kT))
    mass_f = float(np.asarray(mass))
    Q_f = float(np.asarray(Q))
    n_steps = int(np.asarray(n_steps))
    store_every = int(np.asarray(store_every))

    B, D = x0.shape
    B_core = B // N_CORES
    n_chunks = n_steps // store_every

    key = (B_core, D, n_steps, store_every, kT_f, mass_f, Q_f)
    if key not in _BUILD_CACHE:
        _BUILD_CACHE[key] = _build_v3(
            B_core, D, n_steps, store_every, kT_f, mass_f, Q_f
        )
    nc = _BUILD_CACHE[key]

    in_maps = []
    for cc in range(N_CORES):
        sl = slice(cc * B_core, (cc + 1) * B_core)
        in_maps.append(
            {
                "x0": np.ascontiguousarray(x0[sl]),
                "v0": np.ascontiguousarray(v0[sl]),
                "alpha0": np.ascontiguousarray(alpha0[sl]),
            }
        )

    res = run_bass_kernel_spmd(nc, in_maps, core_ids=list(range(N_CORES)))
    results = res.results

    traj_x = np.empty((n_chunks + 1, B, D), np.float32)
    traj_v = np.empty((n_chunks + 1, B, D), np.float32)
    traj_x[0] = x0
    traj_v[0] = v0
    cpi = _pick_cpi(n_chunks, store_every)
    SB = cpi // 2 if cpi % 2 == 0 else 1
    for cc in range(N_CORES):
        sl = slice(cc * B_core, (cc + 1) * B_core)
        # device layout [n_loop*NB, P, SB, G, D] -> host order (g*P + p)
        traj_x[1:, sl] = (
            results[cc]["out_x"]
            .reshape(n_chunks // SB, P, SB, B_core // P, D)
            .transpose(0, 2, 3, 1, 4)
            .reshape(n_chunks, B_core, D)
        )
        traj_v[1:, sl] = (
            results[cc]["out_v"]
            .reshape(n_chunks // SB, P, SB, B_core // P, D)
            .transpose(0, 2, 3, 1, 4)
            .reshape(n_chunks, B_core, D)
        )
    return traj_x, traj_v
